# revision 19
# baseline (speedup 1.0000x reference)
"""CktGNN encoder kernel for Trainium2 (Bass/Tile), 8-core data parallel.

Per core (local batch BL=512 = 4 b-tiles of 128):
  - "L2" tensors: [128 b-partitions, 4*HS free] fp16 (r/z/n/h/Hin/G).
  - "L1" tensors: [hs-partitions, 512 b free] fp16 (transposed h/Hin used as
    matmul stationary operands; produced by PE transpose each step).
  - All matmuls fp16 (1 cyc/row on PE), fp32 PSUM accumulation.
  - Biases folded into matmuls via ones-rows in the stationary data stack.
  - Aggregation Hin_w = sum_u adj[:,w,u] * g_u is split three ways per
    (btile, step): oldest terms -> GPSIMD stt chain, middle terms -> PE as
    diagonal-matmul accumulation (host-prebuilt diag(adj) streamed from HBM),
    newest terms -> DVE stt chain; the guaranteed DAG backbone edge
    (u = w-1, adj == 1) is a plain 2x tensor_tensor add on DVE.
"""
import sys
sys.path.insert(0, "/opt/trn_rl_repo")

import numpy as np
import concourse.bass as bass
import concourse.tile as tile
from concourse import mybir
from concourse.bass_utils import run_bass_kernel_spmd
from concourse.vector_clock import ScopedClock
from contextlib import ExitStack

F16 = mybir.dt.float16
F32 = mybir.dt.float32
AL = mybir.AluOpType
ACTF = mybir.ActivationFunctionType

B = 4096
NCORES = 8
BL = B // NCORES          # 512
NBT = BL // 128           # 4 b-tiles
MAXN = 32
NVT = 26
P9 = 9
XD = NVT + P9             # 35
HS = 301
EMB = 16
FEAT = 8
NZ = 56
FLAT = NBT * HS           # 1204
HALF = 2 * HS             # 602

# XH2 tile rows: [X(35); ones(1); pad(36:64); HinT2(64:109); ones(109)]
XROWS = 110
HNB = 64

# hT2x/head2 tile rows: [hT2(45); pad(45:64); extra(64:74)]
H2ROWS = 74

# hs tiling for transposes
HT = [(0, 128), (128, 256), (256, 301)]

# masked agg term split (oldest->ACT(2-op), middle->PE, newest->DVE)
PE_FRAC = 0.60
ACT_FRAC = 0.25

_patched = [False]


def _patch_tile_drain():
    """This walrus build only supports ONE sem wait on a Drain instruction.
    Split the kernel-tail drain's waits across several drains."""
    if _patched[0]:
        return
    _patched[0] = True

    def patched(self, tick_clock, wait_clock):
        drain_inst = self.nc.sync.drain()
        wait_clock.add_sem_waits(
            drain_inst.ins, ScopedClock({None: tick_clock.global_clock})
        )
        si = drain_inst.ins.sync_info
        waits = list(si.on_wait or [])
        if len(waits) > 1:
            si.on_wait = waits[:1]
            for w in waits[1:]:
                d2 = self.nc.sync.drain()
                si2 = d2.ins.sync_info
                if si2 is None:
                    d2.ins.sync_info = mybir.SyncInfo(on_wait=[w], on_update=[])
                else:
                    si2.on_wait = [w]
        self.nc.all_engine_barrier()
        popped = self.nc._tile_sem_poison_stack.pop()
        assert popped is self._sem_poison
        self.nc.clear_and_free_semaphores(list(self.sems.allocated().values()))
        self.nc.all_engine_barrier()

    tile.TileContext._drain_and_barrier = patched


def _assign(w):
    """Split masked agg terms u in [0, w-2] for target vertex w.
    Returns (act_us, pe_us, dve_us).  The edge u = w-1 is handled separately."""
    us = list(range(w - 1))
    n = len(us)
    npe = int(round(PE_FRAC * n))
    nact = int(round(ACT_FRAC * n))
    act_us = us[:nact]
    pe_us = us[nact:nact + npe]
    dve_us = us[nact + npe:]
    return act_us, pe_us, dve_us


def _pe_terms():
    """Flat ordering of (w, u, c) for the host-built diag tensor."""
    terms = []
    for w in range(2, MAXN):
        _, pe_us, _ = _assign(w)
        for u in pe_us:
            for c in range(NBT):
                terms.append((w, u, c))
    return terms


def _prep_weights(inp):
    f16 = np.float16
    W = {}
    Wg, bg, Wm = inp["Wg"], inp["bg"], inp["Wm"]
    W_ih, W_hh = inp["W_ih"], inp["W_hh"]
    b_ih, b_hh = inp["b_ih"], inp["b_hh"]
    z19 = np.zeros((19, HS), np.float32)

    # gate/mapper moving operands [K, 301]; K-split matches hT0/hT1/hT2x
    # hT2x rows: [h 256:301 (45); pad 19; pos (9); ones (1)] = 74
    W["w_g0"] = Wg[0:128].astype(f16)
    W["w_g1"] = Wg[128:256].astype(f16)
    W["w_g2"] = np.vstack([Wg[256:301], z19, Wg[301:310], bg[None, :]]).astype(f16)
    W["w_m0"] = Wm[0:128].astype(f16)
    W["w_m1"] = Wm[128:256].astype(f16)
    W["w_m2"] = np.vstack(
        [Wm[256:301], z19, Wm[301:310], np.zeros((1, HS), np.float32)]
    ).astype(f16)

    # r,z: rows 0:602.  K-block 0 matches XH2 rows (110):
    # [X(35); ones(35); pad 36:64; HinT2 64:109; ones(109, zero weight)]
    Wih_rz = W_ih[0:2 * HS]
    Whh_rz = W_hh[0:2 * HS]
    b_rz = (b_ih + b_hh)[0:2 * HS]
    W["w_rz0"] = np.vstack([
        Wih_rz.T, b_rz[None, :], np.zeros((28, 2 * HS), np.float32),
        Whh_rz[:, 256:301].T, np.zeros((1, 2 * HS), np.float32),
    ]).astype(f16)                                   # [110, 602]
    W["w_rz1"] = Whh_rz[:, 0:128].T.astype(f16)
    W["w_rz2"] = Whh_rz[:, 128:256].T.astype(f16)

    # hn: rows 602:903.  K-blocks: HinT0, HinT1, xh2[64:110]=[HinT2; ones]
    Whh_n = W_hh[2 * HS:3 * HS]
    b_hhn = b_hh[2 * HS:3 * HS]
    W["w_hn0"] = Whh_n[:, 0:128].T.astype(f16)
    W["w_hn1"] = Whh_n[:, 128:256].T.astype(f16)
    W["w_hnx"] = np.vstack([
        np.zeros((64, HS), np.float32), Whh_n[:, 256:301].T, b_hhn[None, :]
    ]).astype(f16)                                   # [110, 301]; use rows 64:110

    # i_n: K-block [X(35); ones(1)] = xh2[0:36]
    Wih_n = W_ih[2 * HS:3 * HS]
    b_ihn = b_ih[2 * HS:3 * HS]
    W["w_in"] = np.vstack([Wih_n.T, b_ihn[None, :]]).astype(f16)

    # df encoder
    W["w_d1"] = np.vstack([inp["Wd1"], inp["bd1"][None, :]]).astype(f16)  # [28,16]
    W["w_d2"] = np.vstack([inp["Wd2"], inp["bd2"][None, :]]).astype(f16)  # [17,8]

    # head over head2 rows: [h 256:301 (45); pad 19; Hd (8); ones (1)] = 73
    Whead = np.concatenate([inp["Wmu"], inp["Wlv"]], 1)   # [309, 112]
    bhead = np.concatenate([inp["bmu"], inp["blv"]])
    W["w_hd0"] = Whead[0:128].astype(f16)
    W["w_hd1"] = Whead[128:256].astype(f16)
    W["w_hd2"] = np.vstack([
        Whead[256:301], np.zeros((19, 112), np.float32),
        Whead[301:309], bhead[None, :]
    ]).astype(f16)                                   # [73, 112]
    return W


def _build(nc):
    din = {}

    def dram(name, shape, dt, out=False):
        t = nc.dram_tensor(name, list(shape), dt,
                           kind="ExternalOutput" if out else "ExternalInput")
        din[name] = t
        return t

    npe4 = len(_pe_terms())
    xt = dram("xt", [MAXN, HNB + 1, BL], F16)       # rows 0:65 of XH2
    post = dram("post", [MAXN, 10, BL], F16)        # pos one-hot + ones rows
    ones1 = dram("ones1", [1, BL], F16)
    adjt = dram("adjt", [NBT, 128, MAXN * MAXN], F32)
    diag = dram("diag", [max(npe4, 1), 128, 128], F16)
    hdft = dram("hdft", [28, BL], F16)
    ident = dram("ident", [128, 128], F16)
    wnames = [
        ("w_g0", [128, HS]), ("w_g1", [128, HS]), ("w_g2", [H2ROWS, HS]),
        ("w_m0", [128, HS]), ("w_m1", [128, HS]), ("w_m2", [H2ROWS, HS]),
        ("w_rz0", [XROWS, 2 * HS]), ("w_rz1", [128, 2 * HS]),
        ("w_rz2", [128, 2 * HS]),
        ("w_hn0", [128, HS]), ("w_hn1", [128, HS]), ("w_hnx", [XROWS, HS]),
        ("w_in", [36, HS]),
        ("w_d1", [28, EMB]), ("w_d2", [EMB + 1, FEAT]),
        ("w_hd0", [128, 112]), ("w_hd1", [128, 112]), ("w_hd2", [73, 112]),
    ]
    for n, s in wnames:
        dram(n, s, F16)
    out_d = dram("out", [NBT, 128, 112], F32, out=True)

    with tile.TileContext(nc) as tc, ExitStack() as ctx:
        wp = ctx.enter_context(tc.tile_pool(name="w", bufs=1))
        xp = ctx.enter_context(tc.tile_pool(name="x", bufs=3))
        hp = ctx.enter_context(tc.tile_pool(name="h", bufs=2))
        sp = ctx.enter_context(tc.tile_pool(name="s", bufs=2))
        gp_ = ctx.enter_context(tc.tile_pool(name="g", bufs=1))
        dp = ctx.enter_context(tc.tile_pool(name="d", bufs=8))
        pp = ctx.enter_context(tc.tile_pool(name="ps", bufs=2, space="PSUM"))
        ap_ = ctx.enter_context(tc.tile_pool(name="agps", bufs=2, space="PSUM"))

        wt = {}
        for n, s in wnames:
            t = wp.tile(list(s), F16, tag=n, name=f"wt_{n}")
            nc.sync.dma_start(t[:], din[n].ap()[:])
            wt[n] = t
        adj_t = []
        for c in range(NBT):
            t = wp.tile([128, MAXN * MAXN], F32, tag=f"adj{c}", name=f"adj{c}")
            nc.sync.dma_start(t[:], adjt.ap()[c])
            adj_t.append(t)
        id_t = wp.tile([128, 128], F16, tag="ident")
        nc.sync.dma_start(id_t[:], ident.ap()[:])
        hdft_t = wp.tile([28, BL], F16, tag="hdft")
        nc.sync.dma_start(hdft_t[:], hdft.ap()[:])

        g_tiles = [gp_.tile([128, FLAT], F16, tag=f"gv{u}", name=f"gv{u}")
                   for u in range(MAXN - 1)]

        # ---- df encoder ----
        ps_d = pp.tile([128, 1024], F32, tag="ps")
        nc.tensor.matmul(ps_d[0:EMB, 0:BL], wt["w_d1"][:], hdft_t[:],
                         start=True, stop=True)
        relu_t = wp.tile([EMB + 1, BL], F16, tag="relu")
        nc.vector.memset(relu_t[:], 1.0)
        nc.scalar.activation(relu_t[0:EMB, :], ps_d[0:EMB, 0:BL], ACTF.Relu)
        ps_d2 = pp.tile([128, 1024], F32, tag="ps")
        nc.tensor.matmul(ps_d2[0:FEAT, 0:BL], wt["w_d2"][:], relu_t[:],
                         start=True, stop=True)
        hdT_t = wp.tile([FEAT, BL], F16, tag="hdT")
        nc.scalar.activation(hdT_t[:], ps_d2[0:FEAT, 0:BL], ACTF.Copy)

        # ---- per-step helpers ----
        def new_xh2(v):
            t = xp.tile([XROWS, BL], F16, tag="xh2")
            nc.sync.dma_start(t[0:HNB + 1, :], xt.ap()[v])
            nc.sync.dma_start(t[XROWS - 1:XROWS, :], ones1.ap()[:])
            return t

        def pv2(t):
            """[128, 2, 301] view of a [128, 1024] psum tile's two slabs."""
            return t[:].rearrange("p (c w) -> p c w", c=2)[:, :, 0:HS]

        def sb2(t, j):
            """[128, 2, 301] view of half j of a dense [128, FLAT] tile."""
            return t[:, j * HALF:(j + 1) * HALF].rearrange(
                "p (c w) -> p c w", c=2)

        xh2 = new_xh2(0)
        nc.vector.memset(xh2[HNB:XROWS, :], 0.0)   # Hin(0)=0; ones row re-DMAd
        nc.sync.dma_start(xh2[XROWS - 1:XROWS, :], ones1.ap()[:])
        hinT0 = None
        hinT1 = None
        hinL2 = hp.tile([128, FLAT], F16, tag="hinl2")
        nc.vector.memset(hinL2[:], 0.0)

        hT0_last = hT1_last = head2 = None
        pe_flat = 0      # running index into the diag tensor

        for v in range(MAXN):
            last = v == MAXN - 1
            # ---------- GRU matmuls ----------
            ps_in = [pp.tile([128, 1024], F32, tag="ps", name=f"psin{j}_{v}")
                     for j in range(2)]
            for c in range(NBT):
                nc.tensor.matmul(
                    ps_in[c // 2][:, (c % 2) * 512:(c % 2) * 512 + HS],
                    xh2[0:36, bass.ts(c, 128)], wt["w_in"][:],
                    start=True, stop=True)
            in_sb = sp.tile([128, FLAT], F16, tag="insb")
            for j in range(2):
                nc.scalar.activation(sb2(in_sb, j), pv2(ps_in[j]), ACTF.Copy)

            rz_blocks = [(xh2[0:XROWS, :], wt["w_rz0"][:])]
            if v > 0:
                rz_blocks += [(hinT0[:], wt["w_rz1"][:]),
                              (hinT1[:], wt["w_rz2"][:])]
            ps_r = [pp.tile([128, 1024], F32, tag="ps", name=f"psr{j}_{v}")
                    for j in range(2)]
            ps_z = [pp.tile([128, 1024], F32, tag="ps", name=f"psz{j}_{v}")
                    for j in range(2)]
            nk = len(rz_blocks)
            for c in range(NBT):
                dst = ps_r[c // 2][:, (c % 2) * 512:(c % 2) * 512 + HS]
                for k, (st, mv) in enumerate(rz_blocks):
                    nc.tensor.matmul(dst, st[:, bass.ts(c, 128)], mv[:, 0:HS],
                                     start=(k == 0), stop=(k == nk - 1))
            for c in range(NBT):
                dst = ps_z[c // 2][:, (c % 2) * 512:(c % 2) * 512 + HS]
                for k, (st, mv) in enumerate(rz_blocks):
                    nc.tensor.matmul(dst, st[:, bass.ts(c, 128)],
                                     mv[:, HS:2 * HS],
                                     start=(k == 0), stop=(k == nk - 1))
            r_sb = sp.tile([128, FLAT], F16, tag="rsb")
            z_sb = sp.tile([128, FLAT], F16, tag="zsb")
            for j in range(2):
                nc.scalar.activation(sb2(r_sb, j), pv2(ps_r[j]), ACTF.Sigmoid)
            for j in range(2):
                nc.scalar.activation(sb2(z_sb, j), pv2(ps_z[j]), ACTF.Sigmoid)

            hn_blocks = [(xh2[HNB:XROWS, :], wt["w_hnx"][HNB:XROWS, :])]
            if v > 0:
                hn_blocks += [(hinT0[:], wt["w_hn0"][:]),
                              (hinT1[:], wt["w_hn1"][:])]
            ps_hn = [pp.tile([128, 1024], F32, tag="ps", name=f"pshn{j}_{v}")
                     for j in range(2)]
            nk = len(hn_blocks)
            for c in range(NBT):
                dst = ps_hn[c // 2][:, (c % 2) * 512:(c % 2) * 512 + HS]
                for k, (st, mv) in enumerate(hn_blocks):
                    nc.tensor.matmul(dst, st[:, bass.ts(c, 128)], mv[:],
                                     start=(k == 0), stop=(k == nk - 1))
            q_sb = sp.tile([128, FLAT], F16, tag="qsb")
            for j in range(2):
                nc.vector.tensor_tensor(sb2(q_sb, j), sb2(r_sb, j),
                                        pv2(ps_hn[j]), AL.mult)
            t_sb = sp.tile([128, FLAT], F16, tag="tsb")
            nc.vector.tensor_tensor(t_sb[:], q_sb[:], in_sb[:], AL.add)
            n_sb = sp.tile([128, FLAT], F16, tag="nsb")
            nc.scalar.activation(n_sb[:], t_sb[:], ACTF.Tanh)

            d_sb = sp.tile([128, FLAT], F16, tag="dsb")
            nc.vector.tensor_tensor(d_sb[:], hinL2[:], n_sb[:], AL.subtract)
            e_sb = sp.tile([128, FLAT], F16, tag="esb")
            nc.vector.tensor_tensor(e_sb[:], z_sb[:], d_sb[:], AL.mult)
            h_sb = sp.tile([128, FLAT], F16, tag="hsb")
            nc.vector.tensor_tensor(h_sb[:], n_sb[:], e_sb[:], AL.add)

            # ---------- transpose h -> L1 ----------
            tr_h = pp.tile([128, 2048], F16, tag="ps", name=f"trh_{v}")
            for ki, (k0, k1) in enumerate(HT):
                kw = k1 - k0
                for c in range(NBT):
                    nc.tensor.matmul(
                        tr_h[0:kw, ki * 512 + c * 128: ki * 512 + c * 128 + 128],
                        h_sb[:, c * HS + k0: c * HS + k1], id_t[:],
                        is_transpose=True, skip_group_check=True)
            hT0 = hp.tile([128, BL], F16, tag="ht0")
            nc.vector.tensor_copy(hT0[:], tr_h[0:128, 0:512])
            hT1 = hp.tile([128, BL], F16, tag="ht1")
            nc.vector.tensor_copy(hT1[:], tr_h[0:128, 512:1024])
            if not last:
                hT2x = hp.tile([H2ROWS, BL], F16, tag="ht2")
                nc.vector.memset(hT2x[32:HNB, :], 0.0)
                nc.vector.tensor_copy(hT2x[0:45, :], tr_h[0:45, 1024:1536])
                nc.sync.dma_start(hT2x[HNB:H2ROWS, :], post.ap()[v])
            else:
                head2 = hp.tile([73, BL], F16, tag="head2")
                nc.vector.memset(head2[32:HNB, :], 0.0)
                nc.vector.tensor_copy(head2[0:45, :], tr_h[0:45, 1024:1536])
                nc.vector.tensor_copy(head2[HNB:HNB + FEAT, :], hdT_t[:])
                nc.sync.dma_start(head2[72:73, :], ones1.ap()[:])
                hT0_last, hT1_last = hT0, hT1
                break

            # ---------- gate/mapper -> g_v ----------
            ps_gg = [pp.tile([128, 1024], F32, tag="ps", name=f"psgg{j}_{v}")
                     for j in range(2)]
            ps_gm = [pp.tile([128, 1024], F32, tag="ps", name=f"psgm{j}_{v}")
                     for j in range(2)]
            g_blocks = [(hT0, "0"), (hT1, "1"), (hT2x, "2")]
            for c in range(NBT):
                dst = ps_gg[c // 2][:, (c % 2) * 512:(c % 2) * 512 + HS]
                for k, (st, sfx) in enumerate(g_blocks):
                    nc.tensor.matmul(dst, st[:, bass.ts(c, 128)],
                                     wt[f"w_g{sfx}"][:],
                                     start=(k == 0), stop=(k == 2))
            for c in range(NBT):
                dst = ps_gm[c // 2][:, (c % 2) * 512:(c % 2) * 512 + HS]
                for k, (st, sfx) in enumerate(g_blocks):
                    nc.tensor.matmul(dst, st[:, bass.ts(c, 128)],
                                     wt[f"w_m{sfx}"][:],
                                     start=(k == 0), stop=(k == 2))
            sg_sb = sp.tile([128, FLAT], F16, tag="sgsb")
            for j in range(2):
                nc.scalar.activation(sb2(sg_sb, j), pv2(ps_gg[j]),
                                     ACTF.Sigmoid)
            gv = g_tiles[v]
            for j in range(2):
                nc.vector.tensor_tensor(sb2(gv, j), sb2(sg_sb, j),
                                        pv2(ps_gm[j]), AL.mult)

            # ---------- aggregate Hin(w), w = v+1 ----------
            w = v + 1
            act_us, pe_us, dve_us = _assign(w)
            hinL2_next = hp.tile([128, FLAT], F16, tag="hinl2")
            gpart = (hp.tile([128, FLAT], F16, tag="gpart", name="gpart")
                     if act_us else None)
            if pe_us:
                agg_ps = [ap_.tile([128, 1024], F32, tag="agps",
                                   name=f"agg{j}_{v}") for j in range(2)]
                for ui, u in enumerate(pe_us):
                    for c in range(NBT):
                        dg = dp.tile([128, 128], F16, tag="diag", name="dg")
                        nc.sync.dma_start(dg[:], diag.ap()[pe_flat])
                        pe_flat += 1
                        dst = agg_ps[c // 2][:, (c % 2) * 512:(c % 2) * 512 + HS]
                        nc.tensor.matmul(
                            dst, dg[:], g_tiles[u][:, bass.ts(c, HS)],
                            start=(ui == 0), stop=(ui == len(pe_us) - 1))
            else:
                agg_ps = None
            for c in range(NBT):
                hslab = hinL2_next[:, bass.ts(c, HS)]
                first = True
                for u in dve_us:
                    gsl = g_tiles[u][:, bass.ts(c, HS)]
                    sc = adj_t[c][:, w * MAXN + u: w * MAXN + u + 1]
                    if first:
                        nc.vector.tensor_scalar(hslab, gsl, sc, None, AL.mult)
                        first = False
                    else:
                        nc.vector.scalar_tensor_tensor(
                            hslab, gsl, sc, hslab, AL.mult, AL.add)
                # backbone edge u = w-1 (adj == 1 always)
                egsl = g_tiles[w - 1][:, bass.ts(c, HS)]
                if first:
                    nc.vector.tensor_copy(hslab, egsl)
                else:
                    nc.vector.tensor_tensor(hslab, egsl, hslab, AL.add)
                if act_us:
                    gslab = gpart[:, bass.ts(c, HS)]
                    firstg = True
                    for u in act_us:
                        gsl = g_tiles[u][:, bass.ts(c, HS)]
                        sc = adj_t[c][:, w * MAXN + u: w * MAXN + u + 1]
                        if firstg:
                            nc.scalar.mul(gslab, gsl, sc)
                            firstg = False
                        else:
                            mk = sp.tile([128, HS], F16, tag="mask", name="mk")
                            nc.scalar.mul(mk[:], gsl, sc)
                            nc.any.tensor_add(gslab, mk[:], gslab)
            if act_us:
                nc.any.tensor_add(hinL2_next[:], gpart[:], hinL2_next[:])
            if agg_ps is not None:
                for j in range(2):
                    nc.any.tensor_add(sb2(hinL2_next, j), pv2(agg_ps[j]),
                                      sb2(hinL2_next, j))

            # ---------- transpose Hin(w) -> L1; prefetch X(w) ----------
            xh2_next = new_xh2(w)
            tr_n = pp.tile([128, 2048], F16, tag="ps", name=f"trn_{v}")
            for ki, (k0, k1) in enumerate(HT):
                kw = k1 - k0
                for c in range(NBT):
                    nc.tensor.matmul(
                        tr_n[0:kw, ki * 512 + c * 128: ki * 512 + c * 128 + 128],
                        hinL2_next[:, c * HS + k0: c * HS + k1], id_t[:],
                        is_transpose=True, skip_group_check=True)
            hinT0 = hp.tile([128, BL], F16, tag="hinT0")
            nc.vector.tensor_copy(hinT0[:], tr_n[0:128, 0:512])
            hinT1 = hp.tile([128, BL], F16, tag="hinT1")
            nc.vector.tensor_copy(hinT1[:], tr_n[0:128, 512:1024])
            nc.vector.tensor_copy(xh2_next[HNB:HNB + 45, :],
                                  tr_n[0:45, 1024:1536])

            xh2 = xh2_next
            hinL2 = hinL2_next

        # ---------- head ----------
        ps_o = [pp.tile([128, 1024], F32, tag="ps", name=f"pso{j}")
                for j in range(2)]
        hd_blocks = [(hT0_last, "w_hd0"), (hT1_last, "w_hd1"),
                     (head2, "w_hd2")]
        for c in range(NBT):
            dst = ps_o[c // 2][:, (c % 2) * 512:(c % 2) * 512 + 112]
            for k, (st, wn) in enumerate(hd_blocks):
                nc.tensor.matmul(dst, st[:, bass.ts(c, 128)], wt[wn][:],
                                 start=(k == 0), stop=(k == 2))
        out_sb = sp.tile([128, NBT * 112], F32, tag="outsb")
        for j in range(2):
            nc.scalar.activation(
                out_sb[:, j * 224:(j + 1) * 224].rearrange(
                    "p (c w) -> p c w", c=2),
                ps_o[j][:].rearrange("p (c w) -> p c w", c=2)[:, :, 0:112],
                ACTF.Copy)
        nc.sync.dma_start(
            out_d.ap().rearrange("c p w -> p c w"),
            out_sb[:].rearrange("p (c w) -> p c w", c=NBT))
    _cap_sync_waits(nc)
    return nc




def _cap_sync_waits(nc, maxw=2):
    """Walrus codegen in this build supports at most `maxw` sem waits per
    instruction (1 for Drain/NoOp ctrl structs).  Move overflow waits onto
    same-engine NoOp instructions inserted immediately before."""
    fn = nc.m.functions[0]
    nid = [0]
    for bb in fn.blocks:
        insts = list(bb.instructions)
        out = []
        for inst in insts:
            si = inst.sync_info
            waits = list(si.on_wait) if si and si.on_wait else []
            limit = 1
            if len(waits) > limit:
                keep = waits[len(waits) - limit:]
                extra = waits[:len(waits) - limit]
                for w in extra:
                    nop = mybir.InstNoOp(name=f"WCAP-{nid[0]}")
                    nid[0] += 1
                    nop.engine = inst.engine
                    nop.sync_info = mybir.SyncInfo(on_wait=[w], on_update=[])
                    out.append(nop)
                si.on_wait = keep
            out.append(inst)
        bb.instructions = out


def _make_in_maps(inp):
    W = _prep_weights(inp)
    f16 = np.float16

    types_, pos_ = inp["types"], inp["pos"]
    X = np.zeros((B, MAXN, HNB + 1), f16)
    X[np.arange(B)[:, None], np.arange(MAXN)[None, :], types_] = 1
    X[np.arange(B)[:, None], np.arange(MAXN)[None, :], NVT + pos_] = 1
    X[:, :, XD] = 1.0    # ones row for i_n / rz bias

    pos_oh = np.zeros((B, MAXN, 10), f16)
    pos_oh[np.arange(B)[:, None], np.arange(MAXN)[None, :], pos_] = 1
    pos_oh[:, :, 9] = 1.0  # ones row (gate bias)

    adjf = inp["adj"].astype(np.float32)
    hdf_ = inp["hdf"].astype(np.float32)
    terms = _pe_terms()

    in_maps = []
    for core in range(NCORES):
        sl = slice(core * BL, (core + 1) * BL)
        m = {}
        m["xt"] = np.ascontiguousarray(X[sl].transpose(1, 2, 0))
        m["post"] = np.ascontiguousarray(pos_oh[sl].transpose(1, 2, 0))
        m["ones1"] = np.ones((1, BL), f16)
        m["adjt"] = np.ascontiguousarray(adjf[sl].reshape(NBT, 128, MAXN * MAXN))
        adj_core = adjf[sl].reshape(NBT, 128, MAXN, MAXN)
        dg = np.zeros((max(len(terms), 1), 128, 128), f16)
        dgflat = dg.reshape(dg.shape[0], -1)
        for k, (w, u, c) in enumerate(terms):
            dgflat[k, ::129] = adj_core[c, :, w, u]
        m["diag"] = dg
        hd = np.zeros((28, BL), f16)
        hd[0:27] = hdf_[sl].T.astype(f16)
        hd[27] = 1.0
        m["hdft"] = hd
        m["ident"] = np.eye(128, dtype=f16)
        for k, v in W.items():
            m[k] = np.ascontiguousarray(v)
        in_maps.append(m)
    return in_maps


_CACHE = {}


def _get_nc():
    _patch_tile_drain()
    if "nc" not in _CACHE:
        nc = bass.Bass("TRN2", target_bir_lowering=False, debug=False)
        _build(nc)
        _CACHE["nc"] = nc
    return _CACHE["nc"]


def kernel(types, pos, adj, hdf, Wg, bg, Wm, W_ih, W_hh, b_ih, b_hh,
           Wd1, bd1, Wd2, bd2, Wmu, bmu, Wlv, blv, _return_nc=False):
    inp = dict(types=types, pos=pos, adj=adj, hdf=hdf, Wg=Wg, bg=bg, Wm=Wm,
               W_ih=W_ih, W_hh=W_hh, b_ih=b_ih, b_hh=b_hh, Wd1=Wd1, bd1=bd1,
               Wd2=Wd2, bd2=bd2, Wmu=Wmu, bmu=bmu, Wlv=Wlv, blv=blv)
    inp = {k: np.asarray(v) for k, v in inp.items()}
    in_maps = _make_in_maps(inp)
    nc = _get_nc()

    res = run_bass_kernel_spmd(nc, in_maps, list(range(NCORES)))
    mu = np.zeros((B, NZ), np.float32)
    lv = np.zeros((B, NZ), np.float32)
    for core in range(NCORES):
        o = res.results[core]["out"].reshape(BL, 112)
        sl = slice(core * BL, (core + 1) * BL)
        mu[sl] = o[:, 0:NZ]
        lv[sl] = o[:, NZ:112]
    if _return_nc:
        return (mu, lv), res
    return mu, lv


def time_kernel(inp, iters=10):
    """Time on-device execution (min over iters) in ns, without output
    donation so the jitted function can be re-invoked."""
    import time as _time
    import jax
    from jax.sharding import Mesh, PartitionSpec
    from jax.experimental.shard_map import shard_map
    from concourse import bass2jax

    inp = {k: np.asarray(v) for k, v in inp.items()}
    in_maps = _make_in_maps(inp)
    nc = _get_nc()
    bass2jax.install_neuronx_cc_hook()

    partition_name = (nc.partition_id_tensor.name
                      if nc.partition_id_tensor else None)
    in_names, out_names, out_avals, zero_outs = [], [], [], []
    for alloc in nc.m.functions[0].allocations:
        if not isinstance(alloc, mybir.MemoryLocationSet):
            continue
        name = alloc.memorylocations[0].name
        if alloc.kind == "ExternalInput":
            if name != partition_name:
                in_names.append(name)
        elif alloc.kind == "ExternalOutput":
            out_names.append(name)
            shape = tuple(alloc.tensor_shape)
            dtype = mybir.dt.np(alloc.dtype)
            out_avals.append(jax.core.ShapedArray(shape, dtype))
            zero_outs.append(np.zeros(shape, dtype))
    n_params = len(in_names)
    all_in_names = list(in_names) + list(out_names)
    if partition_name is not None:
        all_in_names.append(partition_name)

    def _body(*args):
        operands = list(args)
        if partition_name is not None:
            operands.append(bass2jax.partition_id_tensor())
        outs = bass2jax._bass_exec_p.bind(
            *operands,
            out_avals=tuple(out_avals),
            in_names=tuple(all_in_names),
            out_names=tuple(out_names),
            lowering_input_output_aliases=(),
            sim_require_finite=True,
            sim_require_nnan=True,
            nc=nc,
        )
        return tuple(outs)

    devices = jax.devices()[:NCORES]
    mesh = Mesh(np.asarray(devices), ("core",))
    nin = n_params + len(out_names)
    sharded = jax.jit(
        shard_map(_body, mesh=mesh,
                  in_specs=(PartitionSpec("core"),) * nin,
                  out_specs=(PartitionSpec("core"),) * len(out_names),
                  check_rep=False),
        keep_unused=True,
    )
    concat_in = [np.concatenate([in_maps[c][n] for c in range(NCORES)], 0)
                 for n in in_names]
    concat_zeros = [np.zeros((NCORES * z.shape[0], *z.shape[1:]), z.dtype)
                    for z in zero_outs]
    args = [jax.device_put(a) for a in concat_in + concat_zeros]
    r = sharded(*args)
    jax.block_until_ready(r)
    best = float("inf")
    for _ in range(iters):
        t0 = _time.perf_counter()
        r = sharded(*args)
        jax.block_until_ready(r)
        best = min(best, _time.perf_counter() - t0)
    return best * 1e9


# revision 22
# speedup vs baseline: 137.3543x; 137.3543x over previous
"""CktGNN encoder kernel for Trainium2 (Bass/Tile), 8-core data parallel.

Per core (local batch BL=512 = 4 b-tiles of 128):
  - "L2" tensors: [128 b-partitions, 4*HS free] fp16 (r/z/n/h/Hin/G).
  - "L1" tensors: [hs-partitions, 512 b free] fp16 (transposed h/Hin used as
    matmul stationary operands; produced by PE transpose each step).
  - All matmuls fp16 (1 cyc/row on PE), fp32 PSUM accumulation.
  - Biases folded into matmuls via ones-rows in the stationary data stack.
  - Aggregation Hin_w = sum_u adj[:,w,u] * g_u is split three ways per
    (btile, step): oldest terms -> GPSIMD stt chain, middle terms -> PE as
    diagonal-matmul accumulation (host-prebuilt diag(adj) streamed from HBM),
    newest terms -> DVE stt chain; the guaranteed DAG backbone edge
    (u = w-1, adj == 1) is a plain 2x tensor_tensor add on DVE.
"""
import sys
sys.path.insert(0, "/opt/trn_rl_repo")

import numpy as np
import concourse.bass as bass
import concourse.tile as tile
from concourse import mybir
from concourse.bass_utils import run_bass_kernel_spmd
from concourse.vector_clock import ScopedClock
from contextlib import ExitStack

F16 = mybir.dt.float16
F32 = mybir.dt.float32
AL = mybir.AluOpType
ACTF = mybir.ActivationFunctionType

B = 4096
NCORES = 8
BL = B // NCORES          # 512
NBT = BL // 128           # 4 b-tiles
MAXN = 32
NVT = 26
P9 = 9
XD = NVT + P9             # 35
HS = 301
EMB = 16
FEAT = 8
NZ = 56
FLAT = NBT * HS           # 1204
HALF = 2 * HS             # 602

# XH2 tile rows: [X(35); ones(1); pad(36:64); HinT2(64:109); ones(109)]
XROWS = 110
HNB = 64

# hT2x/head2 tile rows: [hT2(45); pad(45:64); extra(64:74)]
H2ROWS = 74

# hs tiling for transposes
HT = [(0, 128), (128, 256), (256, 301)]

# masked agg term split (oldest->ACT(2-op), middle->PE, newest->DVE)
PE_FRAC = 0.45
ACT_FRAC = 0.0

_patched = [False]


def _patch_tile_drain():
    """This walrus build only supports ONE sem wait on a Drain instruction.
    Split the kernel-tail drain's waits across several drains."""
    if _patched[0]:
        return
    _patched[0] = True

    def patched(self, tick_clock, wait_clock):
        drain_inst = self.nc.sync.drain()
        wait_clock.add_sem_waits(
            drain_inst.ins, ScopedClock({None: tick_clock.global_clock})
        )
        si = drain_inst.ins.sync_info
        waits = list(si.on_wait or [])
        if len(waits) > 1:
            si.on_wait = waits[:1]
            for w in waits[1:]:
                d2 = self.nc.sync.drain()
                si2 = d2.ins.sync_info
                if si2 is None:
                    d2.ins.sync_info = mybir.SyncInfo(on_wait=[w], on_update=[])
                else:
                    si2.on_wait = [w]
        self.nc.all_engine_barrier()
        popped = self.nc._tile_sem_poison_stack.pop()
        assert popped is self._sem_poison
        self.nc.clear_and_free_semaphores(list(self.sems.allocated().values()))
        self.nc.all_engine_barrier()

    tile.TileContext._drain_and_barrier = patched


def _assign(w):
    """Split masked agg terms u in [0, w-2] for target vertex w.
    Returns (act_us, pe_us, dve_us).  The edge u = w-1 is handled separately."""
    us = list(range(w - 1))
    n = len(us)
    npe = int(round(PE_FRAC * n))
    nact = int(round(ACT_FRAC * n))
    act_us = us[:nact]
    pe_us = us[nact:nact + npe]
    dve_us = us[nact + npe:]
    return act_us, pe_us, dve_us


def _pe_terms():
    """Flat ordering of (w, u, c) for the host-built diag tensor."""
    terms = []
    for w in range(2, MAXN):
        _, pe_us, _ = _assign(w)
        for u in pe_us:
            for c in range(NBT):
                terms.append((w, u, c))
    return terms


def _prep_weights(inp):
    f16 = np.float16
    W = {}
    Wg, bg, Wm = inp["Wg"], inp["bg"], inp["Wm"]
    W_ih, W_hh = inp["W_ih"], inp["W_hh"]
    b_ih, b_hh = inp["b_ih"], inp["b_hh"]
    z19 = np.zeros((19, HS), np.float32)

    # gate/mapper moving operands [K, 301]; K-split matches hT0/hT1/hT2x
    # hT2x rows: [h 256:301 (45); pad 19; pos (9); ones (1)] = 74
    W["w_g0"] = Wg[0:128].astype(f16)
    W["w_g1"] = Wg[128:256].astype(f16)
    W["w_g2"] = np.vstack([Wg[256:301], z19, Wg[301:310], bg[None, :]]).astype(f16)
    W["w_m0"] = Wm[0:128].astype(f16)
    W["w_m1"] = Wm[128:256].astype(f16)
    W["w_m2"] = np.vstack(
        [Wm[256:301], z19, Wm[301:310], np.zeros((1, HS), np.float32)]
    ).astype(f16)

    # r,z: rows 0:602.  K-block 0 matches XH2 rows (110):
    # [X(35); ones(35); pad 36:64; HinT2 64:109; ones(109, zero weight)]
    Wih_rz = W_ih[0:2 * HS]
    Whh_rz = W_hh[0:2 * HS]
    b_rz = (b_ih + b_hh)[0:2 * HS]
    W["w_rz0"] = np.vstack([
        Wih_rz.T, b_rz[None, :], np.zeros((28, 2 * HS), np.float32),
        Whh_rz[:, 256:301].T, np.zeros((1, 2 * HS), np.float32),
    ]).astype(f16)                                   # [110, 602]
    W["w_rz1"] = Whh_rz[:, 0:128].T.astype(f16)
    W["w_rz2"] = Whh_rz[:, 128:256].T.astype(f16)

    # hn: rows 602:903.  K-blocks: HinT0, HinT1, xh2[64:110]=[HinT2; ones]
    Whh_n = W_hh[2 * HS:3 * HS]
    b_hhn = b_hh[2 * HS:3 * HS]
    W["w_hn0"] = Whh_n[:, 0:128].T.astype(f16)
    W["w_hn1"] = Whh_n[:, 128:256].T.astype(f16)
    W["w_hnx"] = np.vstack([
        np.zeros((64, HS), np.float32), Whh_n[:, 256:301].T, b_hhn[None, :]
    ]).astype(f16)                                   # [110, 301]; use rows 64:110

    # i_n: K-block [X(35); ones(1)] = xh2[0:36]
    Wih_n = W_ih[2 * HS:3 * HS]
    b_ihn = b_ih[2 * HS:3 * HS]
    W["w_in"] = np.vstack([Wih_n.T, b_ihn[None, :]]).astype(f16)

    # df encoder
    W["w_d1"] = np.vstack([inp["Wd1"], inp["bd1"][None, :]]).astype(f16)  # [28,16]
    W["w_d2"] = np.vstack([inp["Wd2"], inp["bd2"][None, :]]).astype(f16)  # [17,8]

    # head over head2 rows: [h 256:301 (45); pad 19; Hd (8); ones (1)] = 73
    Whead = np.concatenate([inp["Wmu"], inp["Wlv"]], 1)   # [309, 112]
    bhead = np.concatenate([inp["bmu"], inp["blv"]])
    W["w_hd0"] = Whead[0:128].astype(f16)
    W["w_hd1"] = Whead[128:256].astype(f16)
    W["w_hd2"] = np.vstack([
        Whead[256:301], np.zeros((19, 112), np.float32),
        Whead[301:309], bhead[None, :]
    ]).astype(f16)                                   # [73, 112]
    return W


def _build(nc):
    din = {}

    def dram(name, shape, dt, out=False):
        t = nc.dram_tensor(name, list(shape), dt,
                           kind="ExternalOutput" if out else "ExternalInput")
        din[name] = t
        return t

    npe4 = len(_pe_terms())
    xt = dram("xt", [MAXN, HNB + 1, BL], F16)       # rows 0:65 of XH2
    post = dram("post", [MAXN, 10, BL], F16)        # pos one-hot + ones rows
    ones1 = dram("ones1", [1, BL], F16)
    adjt = dram("adjt", [NBT, 128, MAXN * MAXN], F32)
    diag = dram("diag", [max(npe4, 1), 128, 128], F16)
    hdft = dram("hdft", [28, BL], F16)
    ident = dram("ident", [128, 128], F16)
    wnames = [
        ("w_g0", [128, HS]), ("w_g1", [128, HS]), ("w_g2", [H2ROWS, HS]),
        ("w_m0", [128, HS]), ("w_m1", [128, HS]), ("w_m2", [H2ROWS, HS]),
        ("w_rz0", [XROWS, 2 * HS]), ("w_rz1", [128, 2 * HS]),
        ("w_rz2", [128, 2 * HS]),
        ("w_hn0", [128, HS]), ("w_hn1", [128, HS]), ("w_hnx", [XROWS, HS]),
        ("w_in", [36, HS]),
        ("w_d1", [28, EMB]), ("w_d2", [EMB + 1, FEAT]),
        ("w_hd0", [128, 112]), ("w_hd1", [128, 112]), ("w_hd2", [73, 112]),
    ]
    for n, s in wnames:
        dram(n, s, F16)
    out_d = dram("out", [NBT, 128, 112], F32, out=True)

    with tile.TileContext(nc) as tc, ExitStack() as ctx:
        wp = ctx.enter_context(tc.tile_pool(name="w", bufs=1))
        xp = ctx.enter_context(tc.tile_pool(name="x", bufs=3))
        hp = ctx.enter_context(tc.tile_pool(name="h", bufs=2))
        sp = ctx.enter_context(tc.tile_pool(name="s", bufs=2))
        gp_ = ctx.enter_context(tc.tile_pool(name="g", bufs=1))
        dp = ctx.enter_context(tc.tile_pool(name="d", bufs=8))
        pp = ctx.enter_context(tc.tile_pool(name="ps", bufs=2, space="PSUM"))
        ap_ = ctx.enter_context(tc.tile_pool(name="agps", bufs=2, space="PSUM"))

        wt = {}
        for n, s in wnames:
            t = wp.tile(list(s), F16, tag=n, name=f"wt_{n}")
            nc.sync.dma_start(t[:], din[n].ap()[:])
            wt[n] = t
        adj_t = []
        for c in range(NBT):
            t = wp.tile([128, MAXN * MAXN], F32, tag=f"adj{c}", name=f"adj{c}")
            nc.sync.dma_start(t[:], adjt.ap()[c])
            adj_t.append(t)
        id_t = wp.tile([128, 128], F16, tag="ident")
        nc.sync.dma_start(id_t[:], ident.ap()[:])
        hdft_t = wp.tile([28, BL], F16, tag="hdft")
        nc.sync.dma_start(hdft_t[:], hdft.ap()[:])

        g_tiles = [gp_.tile([128, FLAT], F16, tag=f"gv{u}", name=f"gv{u}")
                   for u in range(MAXN - 1)]

        # ---- df encoder ----
        ps_d = pp.tile([128, 1024], F32, tag="ps")
        nc.tensor.matmul(ps_d[0:EMB, 0:BL], wt["w_d1"][:], hdft_t[:],
                         start=True, stop=True)
        relu_t = wp.tile([EMB + 1, BL], F16, tag="relu")
        nc.vector.memset(relu_t[:], 1.0)
        nc.scalar.activation(relu_t[0:EMB, :], ps_d[0:EMB, 0:BL], ACTF.Relu)
        ps_d2 = pp.tile([128, 1024], F32, tag="ps")
        nc.tensor.matmul(ps_d2[0:FEAT, 0:BL], wt["w_d2"][:], relu_t[:],
                         start=True, stop=True)
        hdT_t = wp.tile([FEAT, BL], F16, tag="hdT")
        nc.scalar.activation(hdT_t[:], ps_d2[0:FEAT, 0:BL], ACTF.Copy)

        # ---- per-step helpers ----
        def new_xh2(v):
            t = xp.tile([XROWS, BL], F16, tag="xh2")
            nc.sync.dma_start(t[0:HNB + 1, :], xt.ap()[v])
            nc.sync.dma_start(t[XROWS - 1:XROWS, :], ones1.ap()[:])
            return t

        def pv2(t):
            """[128, 2, 301] view of a [128, 1024] psum tile's two slabs."""
            return t[:].rearrange("p (c w) -> p c w", c=2)[:, :, 0:HS]

        def sb2(t, j):
            """[128, 2, 301] view of half j of a dense [128, FLAT] tile."""
            return t[:, j * HALF:(j + 1) * HALF].rearrange(
                "p (c w) -> p c w", c=2)

        xh2 = new_xh2(0)
        nc.vector.memset(xh2[HNB:XROWS, :], 0.0)   # Hin(0)=0; ones row re-DMAd
        nc.sync.dma_start(xh2[XROWS - 1:XROWS, :], ones1.ap()[:])
        hinT0 = None
        hinT1 = None
        hinL2 = hp.tile([128, FLAT], F16, tag="hinl2")
        nc.vector.memset(hinL2[:], 0.0)

        hT0_last = hT1_last = head2 = None
        pe_flat = 0      # running index into the diag tensor

        for v in range(MAXN):
            last = v == MAXN - 1
            # ---------- GRU matmuls ----------
            ps_in = [pp.tile([128, 1024], F32, tag="ps", name=f"psin{j}_{v}")
                     for j in range(2)]
            for c in range(NBT):
                nc.tensor.matmul(
                    ps_in[c // 2][:, (c % 2) * 512:(c % 2) * 512 + HS],
                    xh2[0:36, bass.ts(c, 128)], wt["w_in"][:],
                    start=True, stop=True)
            in_sb = sp.tile([128, FLAT], F16, tag="insb")
            for j in range(2):
                nc.scalar.activation(sb2(in_sb, j), pv2(ps_in[j]), ACTF.Copy)

            rz_blocks = [(xh2[0:XROWS, :], wt["w_rz0"][:])]
            if v > 0:
                rz_blocks += [(hinT0[:], wt["w_rz1"][:]),
                              (hinT1[:], wt["w_rz2"][:])]
            ps_r = [pp.tile([128, 1024], F32, tag="ps", name=f"psr{j}_{v}")
                    for j in range(2)]
            ps_z = [pp.tile([128, 1024], F32, tag="ps", name=f"psz{j}_{v}")
                    for j in range(2)]
            nk = len(rz_blocks)
            for c in range(NBT):
                dst = ps_r[c // 2][:, (c % 2) * 512:(c % 2) * 512 + HS]
                for k, (st, mv) in enumerate(rz_blocks):
                    nc.tensor.matmul(dst, st[:, bass.ts(c, 128)], mv[:, 0:HS],
                                     start=(k == 0), stop=(k == nk - 1))
            for c in range(NBT):
                dst = ps_z[c // 2][:, (c % 2) * 512:(c % 2) * 512 + HS]
                for k, (st, mv) in enumerate(rz_blocks):
                    nc.tensor.matmul(dst, st[:, bass.ts(c, 128)],
                                     mv[:, HS:2 * HS],
                                     start=(k == 0), stop=(k == nk - 1))
            r_sb = sp.tile([128, FLAT], F16, tag="rsb")
            z_sb = sp.tile([128, FLAT], F16, tag="zsb")
            for j in range(2):
                nc.scalar.activation(sb2(r_sb, j), pv2(ps_r[j]), ACTF.Sigmoid)
            for j in range(2):
                nc.scalar.activation(sb2(z_sb, j), pv2(ps_z[j]), ACTF.Sigmoid)

            hn_blocks = [(xh2[HNB:XROWS, :], wt["w_hnx"][HNB:XROWS, :])]
            if v > 0:
                hn_blocks += [(hinT0[:], wt["w_hn0"][:]),
                              (hinT1[:], wt["w_hn1"][:])]
            ps_hn = [pp.tile([128, 1024], F32, tag="ps", name=f"pshn{j}_{v}")
                     for j in range(2)]
            nk = len(hn_blocks)
            for c in range(NBT):
                dst = ps_hn[c // 2][:, (c % 2) * 512:(c % 2) * 512 + HS]
                for k, (st, mv) in enumerate(hn_blocks):
                    nc.tensor.matmul(dst, st[:, bass.ts(c, 128)], mv[:],
                                     start=(k == 0), stop=(k == nk - 1))
            q_sb = sp.tile([128, FLAT], F16, tag="qsb")
            for j in range(2):
                nc.vector.tensor_tensor(sb2(q_sb, j), sb2(r_sb, j),
                                        pv2(ps_hn[j]), AL.mult)
            t_sb = sp.tile([128, FLAT], F16, tag="tsb")
            nc.vector.tensor_tensor(t_sb[:], q_sb[:], in_sb[:], AL.add)
            n_sb = sp.tile([128, FLAT], F16, tag="nsb")
            nc.scalar.activation(n_sb[:], t_sb[:], ACTF.Tanh)

            d_sb = sp.tile([128, FLAT], F16, tag="dsb")
            nc.vector.tensor_tensor(d_sb[:], hinL2[:], n_sb[:], AL.subtract)
            e_sb = sp.tile([128, FLAT], F16, tag="esb")
            nc.vector.tensor_tensor(e_sb[:], z_sb[:], d_sb[:], AL.mult)
            h_sb = sp.tile([128, FLAT], F16, tag="hsb")
            nc.vector.tensor_tensor(h_sb[:], n_sb[:], e_sb[:], AL.add)

            # ---------- transpose h -> L1 ----------
            tr_h = pp.tile([128, 2048], F16, tag="ps", name=f"trh_{v}")
            for ki, (k0, k1) in enumerate(HT):
                kw = k1 - k0
                for c in range(NBT):
                    nc.tensor.matmul(
                        tr_h[0:kw, ki * 512 + c * 128: ki * 512 + c * 128 + 128],
                        h_sb[:, c * HS + k0: c * HS + k1], id_t[:],
                        is_transpose=True, skip_group_check=True)
            hT0 = hp.tile([128, BL], F16, tag="ht0")
            nc.scalar.copy(hT0[:], tr_h[0:128, 0:512])
            hT1 = hp.tile([128, BL], F16, tag="ht1")
            nc.scalar.copy(hT1[:], tr_h[0:128, 512:1024])
            if not last:
                hT2x = hp.tile([H2ROWS, BL], F16, tag="ht2")
                nc.vector.memset(hT2x[32:HNB, :], 0.0)
                nc.vector.tensor_copy(hT2x[0:45, :], tr_h[0:45, 1024:1536])
                nc.sync.dma_start(hT2x[HNB:H2ROWS, :], post.ap()[v])
            else:
                head2 = hp.tile([73, BL], F16, tag="head2")
                nc.vector.memset(head2[32:HNB, :], 0.0)
                nc.vector.tensor_copy(head2[0:45, :], tr_h[0:45, 1024:1536])
                nc.vector.tensor_copy(head2[HNB:HNB + FEAT, :], hdT_t[:])
                nc.sync.dma_start(head2[72:73, :], ones1.ap()[:])
                hT0_last, hT1_last = hT0, hT1
                break

            # ---------- gate/mapper -> g_v ----------
            ps_gg = [pp.tile([128, 1024], F32, tag="ps", name=f"psgg{j}_{v}")
                     for j in range(2)]
            ps_gm = [pp.tile([128, 1024], F32, tag="ps", name=f"psgm{j}_{v}")
                     for j in range(2)]
            g_blocks = [(hT0, "0"), (hT1, "1"), (hT2x, "2")]
            for c in range(NBT):
                dst = ps_gg[c // 2][:, (c % 2) * 512:(c % 2) * 512 + HS]
                for k, (st, sfx) in enumerate(g_blocks):
                    nc.tensor.matmul(dst, st[:, bass.ts(c, 128)],
                                     wt[f"w_g{sfx}"][:],
                                     start=(k == 0), stop=(k == 2))
            for c in range(NBT):
                dst = ps_gm[c // 2][:, (c % 2) * 512:(c % 2) * 512 + HS]
                for k, (st, sfx) in enumerate(g_blocks):
                    nc.tensor.matmul(dst, st[:, bass.ts(c, 128)],
                                     wt[f"w_m{sfx}"][:],
                                     start=(k == 0), stop=(k == 2))
            sg_sb = sp.tile([128, FLAT], F16, tag="sgsb")
            for j in range(2):
                nc.scalar.activation(sb2(sg_sb, j), pv2(ps_gg[j]),
                                     ACTF.Sigmoid)
            gv = g_tiles[v]
            for j in range(2):
                nc.vector.tensor_tensor(sb2(gv, j), sb2(sg_sb, j),
                                        pv2(ps_gm[j]), AL.mult)

            # ---------- aggregate Hin(w), w = v+1 ----------
            w = v + 1
            act_us, pe_us, dve_us = _assign(w)
            hinL2_next = hp.tile([128, FLAT], F16, tag="hinl2")
            gpart = (hp.tile([128, FLAT], F16, tag="gpart", name="gpart")
                     if act_us else None)
            if pe_us:
                agg_ps = [ap_.tile([128, 1024], F32, tag="agps",
                                   name=f"agg{j}_{v}") for j in range(2)]
                for ui, u in enumerate(pe_us):
                    for c in range(NBT):
                        dg = dp.tile([128, 128], F16, tag="diag", name="dg")
                        nc.sync.dma_start(dg[:], diag.ap()[pe_flat])
                        pe_flat += 1
                        dst = agg_ps[c // 2][:, (c % 2) * 512:(c % 2) * 512 + HS]
                        nc.tensor.matmul(
                            dst, dg[:], g_tiles[u][:, bass.ts(c, HS)],
                            start=(ui == 0), stop=(ui == len(pe_us) - 1))
            else:
                agg_ps = None
            for c in range(NBT):
                hslab = hinL2_next[:, bass.ts(c, HS)]
                first = True
                for u in dve_us:
                    gsl = g_tiles[u][:, bass.ts(c, HS)]
                    sc = adj_t[c][:, w * MAXN + u: w * MAXN + u + 1]
                    if first:
                        nc.vector.tensor_scalar(hslab, gsl, sc, None, AL.mult)
                        first = False
                    else:
                        nc.vector.scalar_tensor_tensor(
                            hslab, gsl, sc, hslab, AL.mult, AL.add)
                # backbone edge u = w-1 (adj == 1 always)
                egsl = g_tiles[w - 1][:, bass.ts(c, HS)]
                if first:
                    nc.vector.tensor_copy(hslab, egsl)
                else:
                    nc.vector.tensor_tensor(hslab, egsl, hslab, AL.add)
                if act_us:
                    gslab = gpart[:, bass.ts(c, HS)]
                    firstg = True
                    for u in act_us:
                        gsl = g_tiles[u][:, bass.ts(c, HS)]
                        sc = adj_t[c][:, w * MAXN + u: w * MAXN + u + 1]
                        if firstg:
                            nc.scalar.mul(gslab, gsl, sc)
                            firstg = False
                        else:
                            mk = sp.tile([128, HS], F16, tag="mask", name="mk")
                            nc.scalar.mul(mk[:], gsl, sc)
                            nc.any.tensor_add(gslab, mk[:], gslab)
            if act_us:
                nc.any.tensor_add(hinL2_next[:], gpart[:], hinL2_next[:])
            if agg_ps is not None:
                for j in range(2):
                    nc.any.tensor_add(sb2(hinL2_next, j), pv2(agg_ps[j]),
                                      sb2(hinL2_next, j))

            # ---------- transpose Hin(w) -> L1; prefetch X(w) ----------
            xh2_next = new_xh2(w)
            tr_n = pp.tile([128, 2048], F16, tag="ps", name=f"trn_{v}")
            for ki, (k0, k1) in enumerate(HT):
                kw = k1 - k0
                for c in range(NBT):
                    nc.tensor.matmul(
                        tr_n[0:kw, ki * 512 + c * 128: ki * 512 + c * 128 + 128],
                        hinL2_next[:, c * HS + k0: c * HS + k1], id_t[:],
                        is_transpose=True, skip_group_check=True)
            hinT0 = hp.tile([128, BL], F16, tag="hinT0")
            nc.scalar.copy(hinT0[:], tr_n[0:128, 0:512])
            hinT1 = hp.tile([128, BL], F16, tag="hinT1")
            nc.scalar.copy(hinT1[:], tr_n[0:128, 512:1024])
            nc.vector.tensor_copy(xh2_next[HNB:HNB + 45, :],
                                  tr_n[0:45, 1024:1536])

            xh2 = xh2_next
            hinL2 = hinL2_next

        # ---------- head ----------
        ps_o = [pp.tile([128, 1024], F32, tag="ps", name=f"pso{j}")
                for j in range(2)]
        hd_blocks = [(hT0_last, "w_hd0"), (hT1_last, "w_hd1"),
                     (head2, "w_hd2")]
        for c in range(NBT):
            dst = ps_o[c // 2][:, (c % 2) * 512:(c % 2) * 512 + 112]
            for k, (st, wn) in enumerate(hd_blocks):
                nc.tensor.matmul(dst, st[:, bass.ts(c, 128)], wt[wn][:],
                                 start=(k == 0), stop=(k == 2))
        out_sb = sp.tile([128, NBT * 112], F32, tag="outsb")
        for j in range(2):
            nc.scalar.activation(
                out_sb[:, j * 224:(j + 1) * 224].rearrange(
                    "p (c w) -> p c w", c=2),
                ps_o[j][:].rearrange("p (c w) -> p c w", c=2)[:, :, 0:112],
                ACTF.Copy)
        nc.sync.dma_start(
            out_d.ap().rearrange("c p w -> p c w"),
            out_sb[:].rearrange("p (c w) -> p c w", c=NBT))
    _cap_sync_waits(nc)
    return nc




def _cap_sync_waits(nc, maxw=2):
    """Walrus codegen in this build supports at most `maxw` sem waits per
    instruction (1 for Drain/NoOp ctrl structs).  Move overflow waits onto
    same-engine NoOp instructions inserted immediately before."""
    fn = nc.m.functions[0]
    nid = [0]
    for bb in fn.blocks:
        insts = list(bb.instructions)
        out = []
        for inst in insts:
            si = inst.sync_info
            waits = list(si.on_wait) if si and si.on_wait else []
            limit = 1
            if len(waits) > limit:
                keep = waits[len(waits) - limit:]
                extra = waits[:len(waits) - limit]
                for w in extra:
                    nop = mybir.InstNoOp(name=f"WCAP-{nid[0]}")
                    nid[0] += 1
                    nop.engine = inst.engine
                    nop.sync_info = mybir.SyncInfo(on_wait=[w], on_update=[])
                    out.append(nop)
                si.on_wait = keep
            out.append(inst)
        bb.instructions = out


def _make_in_maps(inp):
    W = _prep_weights(inp)
    f16 = np.float16

    types_, pos_ = inp["types"], inp["pos"]
    X = np.zeros((B, MAXN, HNB + 1), f16)
    X[np.arange(B)[:, None], np.arange(MAXN)[None, :], types_] = 1
    X[np.arange(B)[:, None], np.arange(MAXN)[None, :], NVT + pos_] = 1
    X[:, :, XD] = 1.0    # ones row for i_n / rz bias

    pos_oh = np.zeros((B, MAXN, 10), f16)
    pos_oh[np.arange(B)[:, None], np.arange(MAXN)[None, :], pos_] = 1
    pos_oh[:, :, 9] = 1.0  # ones row (gate bias)

    adjf = inp["adj"].astype(np.float32)
    hdf_ = inp["hdf"].astype(np.float32)
    terms = _pe_terms()

    in_maps = []
    for core in range(NCORES):
        sl = slice(core * BL, (core + 1) * BL)
        m = {}
        m["xt"] = np.ascontiguousarray(X[sl].transpose(1, 2, 0))
        m["post"] = np.ascontiguousarray(pos_oh[sl].transpose(1, 2, 0))
        m["ones1"] = np.ones((1, BL), f16)
        m["adjt"] = np.ascontiguousarray(adjf[sl].reshape(NBT, 128, MAXN * MAXN))
        adj_core = adjf[sl].reshape(NBT, 128, MAXN, MAXN)
        dg = np.zeros((max(len(terms), 1), 128, 128), f16)
        dgflat = dg.reshape(dg.shape[0], -1)
        for k, (w, u, c) in enumerate(terms):
            dgflat[k, ::129] = adj_core[c, :, w, u]
        m["diag"] = dg
        hd = np.zeros((28, BL), f16)
        hd[0:27] = hdf_[sl].T.astype(f16)
        hd[27] = 1.0
        m["hdft"] = hd
        m["ident"] = np.eye(128, dtype=f16)
        for k, v in W.items():
            m[k] = np.ascontiguousarray(v)
        in_maps.append(m)
    return in_maps


_CACHE = {}


def _get_nc():
    _patch_tile_drain()
    if "nc" not in _CACHE:
        nc = bass.Bass("TRN2", target_bir_lowering=False, debug=False)
        _build(nc)
        _CACHE["nc"] = nc
    return _CACHE["nc"]


def kernel(types, pos, adj, hdf, Wg, bg, Wm, W_ih, W_hh, b_ih, b_hh,
           Wd1, bd1, Wd2, bd2, Wmu, bmu, Wlv, blv, _return_nc=False):
    inp = dict(types=types, pos=pos, adj=adj, hdf=hdf, Wg=Wg, bg=bg, Wm=Wm,
               W_ih=W_ih, W_hh=W_hh, b_ih=b_ih, b_hh=b_hh, Wd1=Wd1, bd1=bd1,
               Wd2=Wd2, bd2=bd2, Wmu=Wmu, bmu=bmu, Wlv=Wlv, blv=blv)
    inp = {k: np.asarray(v) for k, v in inp.items()}
    in_maps = _make_in_maps(inp)
    nc = _get_nc()

    res = run_bass_kernel_spmd(nc, in_maps, list(range(NCORES)))
    mu = np.zeros((B, NZ), np.float32)
    lv = np.zeros((B, NZ), np.float32)
    for core in range(NCORES):
        o = res.results[core]["out"].reshape(BL, 112)
        sl = slice(core * BL, (core + 1) * BL)
        mu[sl] = o[:, 0:NZ]
        lv[sl] = o[:, NZ:112]
    if _return_nc:
        return (mu, lv), res
    return mu, lv


def time_kernel(inp, iters=10):
    """Time on-device execution (min over iters) in ns, without output
    donation so the jitted function can be re-invoked."""
    import time as _time
    import jax
    from jax.sharding import Mesh, PartitionSpec
    from jax.experimental.shard_map import shard_map
    from concourse import bass2jax

    inp = {k: np.asarray(v) for k, v in inp.items()}
    in_maps = _make_in_maps(inp)
    nc = _get_nc()
    bass2jax.install_neuronx_cc_hook()

    partition_name = (nc.partition_id_tensor.name
                      if nc.partition_id_tensor else None)
    in_names, out_names, out_avals, zero_outs = [], [], [], []
    for alloc in nc.m.functions[0].allocations:
        if not isinstance(alloc, mybir.MemoryLocationSet):
            continue
        name = alloc.memorylocations[0].name
        if alloc.kind == "ExternalInput":
            if name != partition_name:
                in_names.append(name)
        elif alloc.kind == "ExternalOutput":
            out_names.append(name)
            shape = tuple(alloc.tensor_shape)
            dtype = mybir.dt.np(alloc.dtype)
            out_avals.append(jax.core.ShapedArray(shape, dtype))
            zero_outs.append(np.zeros(shape, dtype))
    n_params = len(in_names)
    all_in_names = list(in_names) + list(out_names)
    if partition_name is not None:
        all_in_names.append(partition_name)

    def _body(*args):
        operands = list(args)
        if partition_name is not None:
            operands.append(bass2jax.partition_id_tensor())
        outs = bass2jax._bass_exec_p.bind(
            *operands,
            out_avals=tuple(out_avals),
            in_names=tuple(all_in_names),
            out_names=tuple(out_names),
            lowering_input_output_aliases=(),
            sim_require_finite=True,
            sim_require_nnan=True,
            nc=nc,
        )
        return tuple(outs)

    devices = jax.devices()[:NCORES]
    mesh = Mesh(np.asarray(devices), ("core",))
    nin = n_params + len(out_names)
    sharded = jax.jit(
        shard_map(_body, mesh=mesh,
                  in_specs=(PartitionSpec("core"),) * nin,
                  out_specs=(PartitionSpec("core"),) * len(out_names),
                  check_rep=False),
        keep_unused=True,
    )
    concat_in = [np.concatenate([in_maps[c][n] for c in range(NCORES)], 0)
                 for n in in_names]
    concat_zeros = [np.zeros((NCORES * z.shape[0], *z.shape[1:]), z.dtype)
                    for z in zero_outs]
    args = [jax.device_put(a) for a in concat_in + concat_zeros]
    r = sharded(*args)
    jax.block_until_ready(r)
    best = float("inf")
    for _ in range(iters):
        t0 = _time.perf_counter()
        r = sharded(*args)
        jax.block_until_ready(r)
        best = min(best, _time.perf_counter() - t0)
    return best * 1e9
